# revision 6
# baseline (speedup 1.0000x reference)
"""Trainium2 Bass kernel for 2-head causal self-attention.

Problem: embedded [B=4, S=2048, E=1024], Wq/Wk/Wv [H=2, E, HD=512].
out[b, s, h*HD:(h+1)*HD] = softmax(causal(Q K^T / sqrt(HD))) @ V for head h.

Sharding: 8 (b, h) pairs -> 8 cores, one pair each (perfect SPMD balance).

Per-core dataflow (all matmul operands bf16, PSUM f32; 1 cyc/row):
  - Host pre-tiles X^T and W into SBUF-layout DRAM buffers so every DMA
    descriptor has 1-8 KB contiguous per-partition lines (the baseline's
    1KB-line packets capped aggregate DMA at ~72 GB/s and starved the
    pipeline head).
  - Phase 1: QT[d,q], KT[d,q] (W stationary, X^T moving) and V[k,d]
    (X^T stationary, W moving), q/k pipelined in 512-wide chunks.
    V is stored augmented with a ones column at index 256:
    v_aug[k, 0:256]=V[:,0:256], v_aug[k,256]=1, v_aug[k,257:513]=V[:,256:512].
  - Phase 2: scores computed TRANSPOSED: scoresT[k, q-chunk] =
    (KT tile).T @ QT; exp on scalar engine, 0/1 diagonal-block mask on
    vector engine. ctx[q_sub, :] = sum_j attnT[:, q_sub].T @ v_aug[:, j]
    in TWO chains per q-subtile: cols 0:257 (incl. ones column -> the
    softmax denominator lands in psum col 256 as a per-partition scalar)
    and cols 257:513. This removes the baseline's 40 ones-matmul row-sum
    instructions (~17.9k PE cycles) and its 16 rotate matmuls entirely.
  - Strictly-above-diagonal blocks skipped; diagonal-zone score matmuls
    start at column off=128*r (bf16 has no fp32r >=256-width constraint).
  - Output written bf16 (host upcasts); halves output DMA.
"""

import ml_dtypes
import numpy as np

import concourse.bass as bass
import concourse.mybir as mybir
from concourse import bacc
import concourse.tile as tile
from concourse import bass_utils

B, S, E, H, HD = 4, 2048, 1024, 2, 512
P = 128
EO = E // P          # 8 e-tiles (contraction for QKV)
DT = HD // P         # 4 d-tiles (contraction for scores)
NKT = S // P         # 16 k-tiles
NSUP = S // 512      # 4 q super-tiles (512 wide)
HDA = HD + 1         # 513: V augmented with ones column at 256
SCALE = float(HD) ** -0.5
F32 = mybir.dt.float32
EXP = mybir.ActivationFunctionType.Exp
BF16 = mybir.dt.bfloat16

_NC = None


def _body(tc, xt_d, wq_d, wk_d, wv_d, mask_d, out_d):
    nc = tc.nc

    import contextlib

    with contextlib.ExitStack() as ctx:
        per = ctx.enter_context(tc.tile_pool(name="per", bufs=1))
        # Persistent SBUF: QT/KT as [d_inner=128, d_tile, q], V as
        # [k_inner, k_tile, 513], mask diagonal patterns.
        qt = per.tile([P, DT, S], BF16)
        kt = per.tile([P, DT, S], BF16)
        v = per.tile([P, NKT, HDA], BF16)
        mask_sb = per.tile([P, 4, 512], BF16)

        # ones column of v_aug (written once, before any V psum copy)
        nc.gpsimd.memset(v[:, :, 256:257], 1.0)

        # ---------------- Phase 1: QT, KT, V projections ----------------
        # The DMA engines round-robin across ALL outstanding descriptors,
        # so every byte issued early delays the critical first pieces, and
        # the Tile scheduler orders instructions by DATA dependencies only
        # (emission order is NOT preserved). Keep only wq+xc0 (the true
        # head dependency, 2MiB) outstanding at the start; every later DMA
        # is gated by a tiny dummy copy INTO its own destination tile
        # (WAW dep) that reads an early xc piece, so it cannot be hoisted.
        # Matmuls run eo-major (4 open psum chains) so each arriving xc
        # piece feeds 4 matmuls immediately during the DMA-paced head.
        with (
            tc.tile_pool(name="wpool", bufs=1) as wpool,
            tc.tile_pool(name="xpool", bufs=2) as xpool,
            tc.tile_pool(name="ps1", bufs=1, space="PSUM") as ps1,
        ):
            wq_sb = wpool.tile([P, EO, HD], BF16)
            wk_sb = wpool.tile([P, EO, HD], BF16)
            wv_sb = wpool.tile([P, EO, HD], BF16)

            PIECES = ((0, 1), (1, 2), (2, 4), (4, 8))
            for lo, hi in PIECES:
                nc.sync.dma_start(
                    out=wq_sb[:, lo:hi, :], in_=wq_d[:, lo * HD : hi * HD]
                )

            xc0 = xpool.tile([P, EO, 512], BF16, tag="xc", name="xc")
            for lo, hi in PIECES:
                nc.gpsimd.dma_start(
                    out=xc0[:, lo:hi, :], in_=xt_d[0, :, lo * 512 : hi * 512]
                )
            # WAW-gated follow-on DMAs (see note above)
            nc.scalar.copy(wk_sb[:, 0, 0:4], xc0[:, 1, 0:4])
            nc.scalar.dma_start(out=wk_sb, in_=wk_d)
            nc.scalar.copy(wv_sb[:, 0, 0:4], xc0[:, 3, 0:4])
            nc.scalar.dma_start(out=wv_sb, in_=wv_d)
            nc.scalar.copy(mask_sb[:, 0, 0:4], xc0[:, 5, 0:4])
            nc.scalar.dma_start(out=mask_sb, in_=mask_d)

            xc_next = None
            for qc in range(4):  # 512-wide q/k chunk
                xc = xc0 if qc == 0 else xc_next
                if qc < 3:
                    xc_next = xpool.tile([P, EO, 512], BF16, tag="xc", name="xc")
                    nc.scalar.copy(xc_next[:, 0, 0:4], xc[:, 7, 0:4])
                    nc.scalar.dma_start(out=xc_next, in_=xt_d[qc + 1])

                # QT / KT: out[d_tile, q-chunk] = sum_e W[e, d].T @ XT[e, q]
                # QT on psum banks q0-3, KT on k0-3 (no inter-section WAR).
                for w_sb, dst, bank in ((wq_sb, qt, "q"), (wk_sb, kt, "k")):
                    pss4 = [
                        ps1.tile([P, 512], F32, tag=f"{bank}{dm}", name=f"p{dm}")
                        for dm in range(DT)
                    ]
                    for eo in range(EO):
                        for dm in range(DT):
                            nc.tensor.matmul(
                                pss4[dm],
                                lhsT=w_sb[:, eo, dm * P : (dm + 1) * P],
                                rhs=xc[:, eo, :],
                                start=(eo == 0),
                                stop=(eo == EO - 1),
                            )
                    for dm in range(DT):
                        dslice = dst[:, dm, qc * 512 : (qc + 1) * 512]
                        if dm % 2 == 0:
                            nc.scalar.copy(dslice, pss4[dm])
                        else:
                            nc.vector.tensor_copy(dslice, pss4[dm])

                # V: out[k_tile, d] = sum_e XT[e, k].T @ Wv[e, d]
                # psum cols 0:256 -> v[:, kg, 0:256]; 256:512 -> 257:513
                psv4 = [
                    ps1.tile([P, 512], F32, tag=f"q{ki}", name=f"pv{ki}")
                    for ki in range(4)
                ]
                for eo in range(EO):
                    for ki in range(4):
                        nc.tensor.matmul(
                            psv4[ki],
                            lhsT=xc[:, eo, ki * P : (ki + 1) * P],
                            rhs=wv_sb[:, eo, :],
                            start=(eo == 0),
                            stop=(eo == EO - 1),
                        )
                for ki in range(4):
                    kg = qc * 4 + ki
                    nc.vector.tensor_copy(v[:, kg, 0:256], psv4[ki][:, 0:256])
                    nc.scalar.copy(v[:, kg, 257:513], psv4[ki][:, 256:512])

        # ---------------- Phase 2: attention ----------------
        with (
            tc.tile_pool(name="apool", bufs=2) as apool,
            tc.tile_pool(name="opool", bufs=2) as opool,
            tc.tile_pool(name="pss", bufs=3, space="PSUM") as pss,
            tc.tile_pool(name="psc", bufs=1, space="PSUM") as psc,
        ):
            for M in range(NSUP):  # q super-tile: q in [512M, 512(M+1))
                at = apool.tile([P, NKT, 512], BF16, tag="at")
                njt = 4 * M + 4  # causal: k-tiles 0 .. 4M+3
                for j in range(njt):
                    r = j - 4 * M
                    # Diagonal-zone tiles: cols < 128r are fully masked.
                    off = P * r if r > 0 else 0
                    ps = pss.tile([P, 512], F32, tag="s")
                    for dt_i in range(DT):
                        nc.tensor.matmul(
                            ps[:, off:512],
                            lhsT=kt[:, dt_i, j * P : (j + 1) * P],
                            rhs=qt[:, dt_i, M * 512 + off : (M + 1) * 512],
                            start=(dt_i == 0),
                            stop=(dt_i == DT - 1),
                        )
                    a_j = at[:, j, off:512]
                    # attnT[k, q] = exp(scoresT / sqrt(hd)); masking after.
                    nc.scalar.activation(a_j, ps[:, off:512], EXP, scale=SCALE)
                    if r >= 0:  # diagonal-zone: zero invalid (q < k) cols
                        nc.vector.tensor_mul(a_j, a_j, mask_sb[:, r, off:512])

                # ctx pass A: cols 0:257 of v_aug (incl. ones column ->
                # denominator in psum col 256). s ascending so s=0 (which
                # doesn't need the last diagonal attn tile) issues first.
                o_sbs = []
                rinvs = []
                for s in range(4):
                    cps = psc.tile([P, 512], F32, tag=f"c{s}", name=f"c{s}")
                    nj = 4 * M + s + 1
                    for j in range(nj):
                        nc.tensor.matmul(
                            cps[:, 0:257],
                            lhsT=at[:, j, s * P : (s + 1) * P],
                            rhs=v[:, j, 0:257],
                            start=(j == 0),
                            stop=(j == nj - 1),
                        )
                    rinv = opool.tile([P, 1], F32, tag=f"r{s}")
                    nc.vector.reciprocal(rinv, cps[:, 256:257])
                    o_sb = opool.tile([P, HD], BF16, tag=f"o{s}")
                    nc.vector.tensor_scalar_mul(
                        o_sb[:, 0:256], cps[:, 0:256], rinv
                    )
                    o_sbs.append(o_sb)
                    rinvs.append(rinv)

                # ctx pass B: cols 257:513 (HD cols 256:512); reuses the
                # same psum banks after pass A's reads complete.
                for s in range(4):
                    cps = psc.tile([P, 512], F32, tag=f"c{s}", name=f"c{s}")
                    nj = 4 * M + s + 1
                    for j in range(nj):
                        nc.tensor.matmul(
                            cps[:, 0:256],
                            lhsT=at[:, j, s * P : (s + 1) * P],
                            rhs=v[:, j, 257:513],
                            start=(j == 0),
                            stop=(j == nj - 1),
                        )
                    nc.vector.tensor_scalar_mul(
                        o_sbs[s][:, 256:512], cps[:, 0:256], rinvs[s]
                    )
                    row0 = M * 512 + s * P
                    nc.sync.dma_start(
                        out=out_d[row0 : row0 + P, :], in_=o_sbs[s]
                    )


def _build_nc():
    nc = bacc.Bacc("TRN2", target_bir_lowering=False, debug=False, num_devices=8)
    # Host-pre-tiled layouts: per-partition lines are contiguous.
    xt_d = nc.dram_tensor("xt", [4, P, EO * 512], BF16, kind="ExternalInput")
    wq_d = nc.dram_tensor("wq", [P, EO * HD], BF16, kind="ExternalInput")
    wk_d = nc.dram_tensor("wk", [P, EO * HD], BF16, kind="ExternalInput")
    wv_d = nc.dram_tensor("wv", [P, EO * HD], BF16, kind="ExternalInput")
    mask_d = nc.dram_tensor("mask", [P, 4, 512], BF16, kind="ExternalInput")
    out_d = nc.dram_tensor("out", [S, HD], BF16, kind="ExternalOutput")
    with tile.TileContext(nc) as tc:
        _body(tc, xt_d.ap(), wq_d.ap(), wk_d.ap(), wv_d.ap(), mask_d.ap(), out_d.ap())
    nc.compile()
    return nc


def _mask_np():
    # mask[p][r, q_local] = 1 iff q_local >= 128*r + k_local(p)
    q = np.arange(512)[None, :]
    k = np.arange(P)[:, None]
    m = np.stack(
        [(q >= (P * r + k)).astype(np.float32) for r in range(4)], axis=0
    )  # [4, 128, 512]
    return np.ascontiguousarray(m.transpose(1, 0, 2))  # [128, 4, 512]


def _in_maps(embedded, Wq, Wk, Wv):
    embedded = np.asarray(embedded, dtype=np.float32)
    mask = _mask_np().astype(ml_dtypes.bfloat16)

    def tile_w(w):  # [E, HD] -> [P, EO*HD] with [p][eo*512+d] layout
        w = np.asarray(w, dtype=np.float32).astype(ml_dtypes.bfloat16)
        return np.ascontiguousarray(
            w.reshape(EO, P, HD).transpose(1, 0, 2).reshape(P, EO * HD)
        )

    def tile_x(x):  # [S, E] -> [4, P, EO*512]: [qc][p][eo*512+col]
        x = np.asarray(x, dtype=np.float32).astype(ml_dtypes.bfloat16)
        return np.ascontiguousarray(
            x.reshape(4, 512, EO, P).transpose(0, 3, 2, 1).reshape(4, P, EO * 512)
        )

    in_maps = []
    for core in range(8):
        b, h = divmod(core, 2)
        in_maps.append(
            {
                "xt": tile_x(embedded[b]),
                "wq": tile_w(Wq[h]),
                "wk": tile_w(Wk[h]),
                "wv": tile_w(Wv[h]),
                "mask": mask,
            }
        )
    return in_maps


def _gather(results):
    out = np.empty((B, S, H * HD), np.float32)
    for core in range(8):
        b, h = divmod(core, 2)
        out[b, :, h * HD : (h + 1) * HD] = np.asarray(
            results[core]["out"]
        ).astype(np.float32)
    return out


def _get_nc():
    global _NC
    if _NC is None:
        _NC = _build_nc()
    return _NC


def kernel(embedded, Wq, Wk, Wv):
    res = bass_utils.run_bass_kernel_spmd(
        _get_nc(), _in_maps(embedded, Wq, Wk, Wv), core_ids=list(range(8))
    )
    return _gather(res.results)


def kernel_traced(embedded, Wq, Wk, Wv):
    """Like kernel() but with NTFF tracing; returns (out, BassKernelResults)."""
    res = bass_utils.run_bass_kernel_spmd(
        _get_nc(), _in_maps(embedded, Wq, Wk, Wv), core_ids=list(range(8)), trace=True
    )
    return _gather(res.results), res


# revision 8
# speedup vs baseline: 1.1587x; 1.1587x over previous
"""Trainium2 Bass kernel for 2-head causal self-attention.

Problem: embedded [B=4, S=2048, E=1024], Wq/Wk/Wv [H=2, E, HD=512].
out[b, s, h*HD:(h+1)*HD] = softmax(causal(Q K^T / sqrt(HD))) @ V for head h.

Sharding: 8 (b, h) pairs -> 8 cores, one pair each (perfect SPMD balance).

Per-core dataflow (all matmul operands bf16, PSUM f32; 1 cyc/row):
  - Host pre-tiles X^T and W into SBUF-layout DRAM buffers so every DMA
    descriptor has 1-8 KB contiguous per-partition lines (the baseline's
    1KB-line packets capped aggregate DMA at ~72 GB/s and starved the
    pipeline head).
  - Phase 1: QT[d,q], KT[d,q] (W stationary, X^T moving) and V[k,d]
    (X^T stationary, W moving), q/k pipelined in 512-wide chunks.
    V is stored augmented with a ones column at index 256:
    v_aug[k, 0:256]=V[:,0:256], v_aug[k,256]=1, v_aug[k,257:513]=V[:,256:512].
  - Phase 2: scores computed TRANSPOSED: scoresT[k, q-chunk] =
    (KT tile).T @ QT; exp on scalar engine, 0/1 diagonal-block mask on
    vector engine. ctx[q_sub, :] = sum_j attnT[:, q_sub].T @ v_aug[:, j]
    in TWO chains per q-subtile: cols 0:257 (incl. ones column -> the
    softmax denominator lands in psum col 256 as a per-partition scalar)
    and cols 257:513. This removes the baseline's 40 ones-matmul row-sum
    instructions (~17.9k PE cycles) and its 16 rotate matmuls entirely.
  - Strictly-above-diagonal blocks skipped; diagonal-zone score matmuls
    start at column off=128*r (bf16 has no fp32r >=256-width constraint).
  - Output written bf16 (host upcasts); halves output DMA.
"""

import ml_dtypes
import numpy as np

import concourse.bass as bass
import concourse.mybir as mybir
from concourse import bacc
import concourse.tile as tile
from concourse import bass_utils

B, S, E, H, HD = 4, 2048, 1024, 2, 512
P = 128
EO = E // P          # 8 e-tiles (contraction for QKV)
DT = HD // P         # 4 d-tiles (contraction for scores)
NKT = S // P         # 16 k-tiles
NSUP = S // 512      # 4 q super-tiles (512 wide)
HDA = HD + 1         # 513: V augmented with ones column at 256
SCALE = float(HD) ** -0.5
F32 = mybir.dt.float32
EXP = mybir.ActivationFunctionType.Exp
BF16 = mybir.dt.bfloat16

_NC = None


def _body(tc, xt_d, wq_d, wk_d, wv_d, mask_d, out_d):
    nc = tc.nc

    import contextlib

    with contextlib.ExitStack() as ctx:
        per = ctx.enter_context(tc.tile_pool(name="per", bufs=1))
        # Persistent SBUF: QT/KT as [d_inner=128, d_tile, q], V as
        # [k_inner, k_tile, 513], mask diagonal patterns.
        qt = per.tile([P, DT, S], BF16)
        kt = per.tile([P, DT, S], BF16)
        v = per.tile([P, NKT, HDA], BF16)
        mask_sb = per.tile([P, 4, 512], BF16)

        # ones column of v_aug (written once, before any V psum copy).
        # On vector, NOT gpsimd: gpsimd must issue the first xc DMA
        # descriptors as early as possible.
        nc.vector.memset(v[:, :, 256:257], 1.0)

        # ---------------- Phase 1: QT, KT, V projections ----------------
        # The DMA engines round-robin across ALL outstanding descriptors,
        # so every byte issued early delays the critical first pieces, and
        # the Tile scheduler orders instructions by DATA dependencies only
        # (emission order is NOT preserved). Keep only wq+xc0 (the true
        # head dependency, 2MiB) outstanding at the start; every later DMA
        # is gated by a tiny dummy copy INTO its own destination tile
        # (WAW dep) that reads an early xc piece, so it cannot be hoisted.
        # Matmuls run eo-major (4 open psum chains) so each arriving xc
        # piece feeds 4 matmuls immediately during the DMA-paced head.
        with (
            tc.tile_pool(name="wpool", bufs=1) as wpool,
            tc.tile_pool(name="xpool", bufs=2) as xpool,
            tc.tile_pool(name="ps1", bufs=1, space="PSUM") as ps1,
        ):
            wq_sb = wpool.tile([P, EO, HD], BF16)
            wk_sb = wpool.tile([P, EO, HD], BF16)
            wv_sb = wpool.tile([P, EO, HD], BF16)

            PIECES = ((0, 1), (1, 2), (2, 4), (4, 8))
            for lo, hi in PIECES:
                nc.sync.dma_start(
                    out=wq_sb[:, lo:hi, :], in_=wq_d[:, lo * HD : hi * HD]
                )

            xc0 = xpool.tile([P, EO, 512], BF16, tag="xc", name="xc")
            for lo, hi in PIECES:
                nc.gpsimd.dma_start(
                    out=xc0[:, lo:hi, :], in_=xt_d[0, :, lo * 512 : hi * 512]
                )
            # WAW-gated follow-on DMAs (see note above)
            nc.scalar.copy(wk_sb[:, 0, 0:4], xc0[:, 1, 0:4])
            nc.scalar.dma_start(out=wk_sb, in_=wk_d)
            nc.scalar.copy(wv_sb[:, 0, 0:4], xc0[:, 3, 0:4])
            nc.scalar.dma_start(out=wv_sb, in_=wv_d)
            nc.scalar.copy(mask_sb[:, 0, 0:4], xc0[:, 5, 0:4])
            nc.scalar.dma_start(out=mask_sb, in_=mask_d)

            xc_next = None
            for qc in range(4):  # 512-wide q/k chunk
                xc = xc0 if qc == 0 else xc_next
                if qc < 3:
                    xc_next = xpool.tile([P, EO, 512], BF16, tag="xc", name="xc")
                    nc.scalar.copy(xc_next[:, 0, 0:4], xc[:, 7, 0:4])
                    nc.scalar.dma_start(out=xc_next, in_=xt_d[qc + 1])

                # QT / KT: out[d_tile, q-chunk] = sum_e W[e, d].T @ XT[e, q]
                # dm-major chains: consecutive matmuls share a psum bank
                # (back-to-back matmuls to different banks run ~20% slower).
                for w_sb, dst, eng in ((wq_sb, qt, "s"), (wk_sb, kt, "v")):
                    for dm in range(DT):
                        ps = ps1.tile([P, 512], F32, tag="ps", bufs=6)
                        for eo in range(EO):
                            nc.tensor.matmul(
                                ps,
                                lhsT=w_sb[:, eo, dm * P : (dm + 1) * P],
                                rhs=xc[:, eo, :],
                                start=(eo == 0),
                                stop=(eo == EO - 1),
                            )
                        dslice = dst[:, dm, qc * 512 : (qc + 1) * 512]
                        if eng == "s":
                            nc.scalar.copy(dslice, ps)
                        else:
                            nc.vector.tensor_copy(dslice, ps)

                # V: out[k_tile, d] = sum_e XT[e, k].T @ Wv[e, d]
                # psum cols 0:256 -> v[:, kg, 0:256]; 256:512 -> 257:513
                for ki in range(4):
                    kg = qc * 4 + ki
                    ps = ps1.tile([P, 512], F32, tag="ps", bufs=6)
                    for eo in range(EO):
                        nc.tensor.matmul(
                            ps,
                            lhsT=xc[:, eo, ki * P : (ki + 1) * P],
                            rhs=wv_sb[:, eo, :],
                            start=(eo == 0),
                            stop=(eo == EO - 1),
                        )
                    nc.vector.tensor_copy(v[:, kg, 0:256], ps[:, 0:256])
                    nc.scalar.copy(v[:, kg, 257:513], ps[:, 256:512])

        # ---------------- Phase 2: attention ----------------
        with (
            tc.tile_pool(name="apool", bufs=2) as apool,
            tc.tile_pool(name="opool", bufs=2) as opool,
            tc.tile_pool(name="pss", bufs=3, space="PSUM") as pss,
            tc.tile_pool(name="psc", bufs=1, space="PSUM") as psc,
        ):
            for M in range(NSUP):  # q super-tile: q in [512M, 512(M+1))
                at = apool.tile([P, NKT, 512], BF16, tag="at")
                njt = 4 * M + 4  # causal: k-tiles 0 .. 4M+3
                for j in range(njt):
                    r = j - 4 * M
                    # Diagonal-zone tiles: cols < 128r are fully masked.
                    off = P * r if r > 0 else 0
                    ps = pss.tile([P, 512], F32, tag="s")
                    for dt_i in range(DT):
                        nc.tensor.matmul(
                            ps[:, off:512],
                            lhsT=kt[:, dt_i, j * P : (j + 1) * P],
                            rhs=qt[:, dt_i, M * 512 + off : (M + 1) * 512],
                            start=(dt_i == 0),
                            stop=(dt_i == DT - 1),
                        )
                    a_j = at[:, j, off:512]
                    # attnT[k, q] = exp(scoresT / sqrt(hd)); masking after.
                    nc.scalar.activation(a_j, ps[:, off:512], EXP, scale=SCALE)
                    if r >= 0:  # diagonal-zone: zero invalid (q < k) cols
                        nc.vector.tensor_mul(a_j, a_j, mask_sb[:, r, off:512])

                # ctx pass A: cols 0:257 of v_aug (incl. ones column ->
                # denominator in psum col 256). s ascending so s=0 (which
                # doesn't need the last diagonal attn tile) issues first.
                o_sbs = []
                rinvs = []
                for s in range(4):
                    cps = psc.tile([P, 512], F32, tag=f"c{s}", name=f"c{s}")
                    nj = 4 * M + s + 1
                    for j in range(nj):
                        nc.tensor.matmul(
                            cps[:, 0:257],
                            lhsT=at[:, j, s * P : (s + 1) * P],
                            rhs=v[:, j, 0:257],
                            start=(j == 0),
                            stop=(j == nj - 1),
                        )
                    rinv = opool.tile([P, 1], F32, tag=f"r{s}")
                    nc.vector.reciprocal(rinv, cps[:, 256:257])
                    o_sb = opool.tile([P, HD], BF16, tag=f"o{s}")
                    nc.vector.tensor_scalar_mul(
                        o_sb[:, 0:256], cps[:, 0:256], rinv
                    )
                    o_sbs.append(o_sb)
                    rinvs.append(rinv)

                # ctx pass B: cols 257:513 (HD cols 256:512); reuses the
                # same psum banks after pass A's reads complete.
                for s in range(4):
                    cps = psc.tile([P, 512], F32, tag=f"c{s}", name=f"c{s}")
                    nj = 4 * M + s + 1
                    for j in range(nj):
                        nc.tensor.matmul(
                            cps[:, 0:256],
                            lhsT=at[:, j, s * P : (s + 1) * P],
                            rhs=v[:, j, 257:513],
                            start=(j == 0),
                            stop=(j == nj - 1),
                        )
                    nc.vector.tensor_scalar_mul(
                        o_sbs[s][:, 256:512], cps[:, 0:256], rinvs[s]
                    )
                    row0 = M * 512 + s * P
                    nc.sync.dma_start(
                        out=out_d[row0 : row0 + P, :], in_=o_sbs[s]
                    )


def _build_nc():
    nc = bacc.Bacc("TRN2", target_bir_lowering=False, debug=False, num_devices=8)
    # Host-pre-tiled layouts: per-partition lines are contiguous.
    xt_d = nc.dram_tensor("xt", [4, P, EO * 512], BF16, kind="ExternalInput")
    wq_d = nc.dram_tensor("wq", [P, EO * HD], BF16, kind="ExternalInput")
    wk_d = nc.dram_tensor("wk", [P, EO * HD], BF16, kind="ExternalInput")
    wv_d = nc.dram_tensor("wv", [P, EO * HD], BF16, kind="ExternalInput")
    mask_d = nc.dram_tensor("mask", [P, 4, 512], BF16, kind="ExternalInput")
    out_d = nc.dram_tensor("out", [S, HD], BF16, kind="ExternalOutput")
    with tile.TileContext(nc) as tc:
        _body(tc, xt_d.ap(), wq_d.ap(), wk_d.ap(), wv_d.ap(), mask_d.ap(), out_d.ap())
    nc.compile()
    return nc


def _mask_np():
    # mask[p][r, q_local] = 1 iff q_local >= 128*r + k_local(p)
    q = np.arange(512)[None, :]
    k = np.arange(P)[:, None]
    m = np.stack(
        [(q >= (P * r + k)).astype(np.float32) for r in range(4)], axis=0
    )  # [4, 128, 512]
    return np.ascontiguousarray(m.transpose(1, 0, 2))  # [128, 4, 512]


def _in_maps(embedded, Wq, Wk, Wv):
    embedded = np.asarray(embedded, dtype=np.float32)
    mask = _mask_np().astype(ml_dtypes.bfloat16)

    def tile_w(w):  # [E, HD] -> [P, EO*HD] with [p][eo*512+d] layout
        w = np.asarray(w, dtype=np.float32).astype(ml_dtypes.bfloat16)
        return np.ascontiguousarray(
            w.reshape(EO, P, HD).transpose(1, 0, 2).reshape(P, EO * HD)
        )

    def tile_x(x):  # [S, E] -> [4, P, EO*512]: [qc][p][eo*512+col]
        x = np.asarray(x, dtype=np.float32).astype(ml_dtypes.bfloat16)
        return np.ascontiguousarray(
            x.reshape(4, 512, EO, P).transpose(0, 3, 2, 1).reshape(4, P, EO * 512)
        )

    in_maps = []
    for core in range(8):
        b, h = divmod(core, 2)
        in_maps.append(
            {
                "xt": tile_x(embedded[b]),
                "wq": tile_w(Wq[h]),
                "wk": tile_w(Wk[h]),
                "wv": tile_w(Wv[h]),
                "mask": mask,
            }
        )
    return in_maps


def _gather(results):
    out = np.empty((B, S, H * HD), np.float32)
    for core in range(8):
        b, h = divmod(core, 2)
        out[b, :, h * HD : (h + 1) * HD] = np.asarray(
            results[core]["out"]
        ).astype(np.float32)
    return out


def _get_nc():
    global _NC
    if _NC is None:
        _NC = _build_nc()
    return _NC


def kernel(embedded, Wq, Wk, Wv):
    res = bass_utils.run_bass_kernel_spmd(
        _get_nc(), _in_maps(embedded, Wq, Wk, Wv), core_ids=list(range(8))
    )
    return _gather(res.results)


def kernel_traced(embedded, Wq, Wk, Wv):
    """Like kernel() but with NTFF tracing; returns (out, BassKernelResults)."""
    res = bass_utils.run_bass_kernel_spmd(
        _get_nc(), _in_maps(embedded, Wq, Wk, Wv), core_ids=list(range(8)), trace=True
    )
    return _gather(res.results), res


# revision 13
# speedup vs baseline: 1.1657x; 1.0061x over previous
"""Trainium2 Bass kernel for 2-head causal self-attention.

Problem: embedded [B=4, S=2048, E=1024], Wq/Wk/Wv [H=2, E, HD=512].
out[b, s, h*HD:(h+1)*HD] = softmax(causal(Q K^T / sqrt(HD))) @ V for head h.

Sharding: 8 (b, h) pairs -> 8 cores, one pair each (perfect SPMD balance).

Per-core dataflow (all matmul operands bf16, PSUM f32; 1 cyc/row):
  - Host pre-tiles X^T and W into SBUF-layout DRAM buffers so every DMA
    descriptor has 1-8 KB contiguous per-partition lines (the baseline's
    1KB-line packets capped aggregate DMA at ~72 GB/s and starved the
    pipeline head).
  - Phase 1: QT[d,q], KT[d,q] (W stationary, X^T moving) and V[k,d]
    (X^T stationary, W moving), q/k pipelined in 512-wide chunks.
    V is stored augmented with a ones column at index 256:
    v_aug[k, 0:256]=V[:,0:256], v_aug[k,256]=1, v_aug[k,257:513]=V[:,256:512].
  - Phase 2: scores computed TRANSPOSED: scoresT[k, q-chunk] =
    (KT tile).T @ QT; exp on scalar engine, 0/1 diagonal-block mask on
    vector engine. ctx[q_sub, :] = sum_j attnT[:, q_sub].T @ v_aug[:, j]
    in TWO chains per q-subtile: cols 0:257 (incl. ones column -> the
    softmax denominator lands in psum col 256 as a per-partition scalar)
    and cols 257:513. This removes the baseline's 40 ones-matmul row-sum
    instructions (~17.9k PE cycles) and its 16 rotate matmuls entirely.
  - Strictly-above-diagonal blocks skipped; diagonal-zone score matmuls
    start at column off=128*r (bf16 has no fp32r >=256-width constraint).
  - Output written bf16 (host upcasts); halves output DMA.
"""

import ml_dtypes
import numpy as np

import concourse.bass as bass
import concourse.mybir as mybir
from concourse import bacc
import concourse.tile as tile
from concourse import bass_utils

B, S, E, H, HD = 4, 2048, 1024, 2, 512
P = 128
EO = E // P          # 8 e-tiles (contraction for QKV)
DT = HD // P         # 4 d-tiles (contraction for scores)
NKT = S // P         # 16 k-tiles
NSUP = S // 512      # 4 q super-tiles (512 wide)
HDA = HD + 1         # 513: V augmented with ones column at 256
SCALE = float(HD) ** -0.5
F32 = mybir.dt.float32
EXP = mybir.ActivationFunctionType.Exp
BF16 = mybir.dt.bfloat16

_NC = None


def _body(tc, xt_d, wq_d, wk_d, wv_d, mask_d, out_d):
    nc = tc.nc

    import contextlib

    with contextlib.ExitStack() as ctx:
        per = ctx.enter_context(tc.tile_pool(name="per", bufs=1))
        # Persistent SBUF: QT/KT as [d_inner=128, d_tile, q], V as
        # [k_inner, k_tile, 513], mask diagonal patterns.
        qt = per.tile([P, DT, S], BF16)
        kt = per.tile([P, DT, S], BF16)
        v = per.tile([P, NKT, HDA], BF16)
        mask_sb = per.tile([P, 4, 512], BF16)

        # ones column of v_aug (written once, before any V psum copy).
        # On vector, NOT gpsimd: gpsimd must issue the first xc DMA
        # descriptors as early as possible.
        nc.vector.memset(v[:, :, 256:257], 1.0)

        # Scores psum pool opened FIRST so it gets PSUM banks phase 1
        # never touches: phase 2's first scores chain then has no
        # write-after-read wait on a phase-1 bank.
        pss = ctx.enter_context(tc.tile_pool(name="pss", bufs=2, space="PSUM"))

        # PE p-state warmup: the tensor engine clock ramps with sustained
        # use, and the first real matmuls otherwise run ~60% slower while
        # also being DMA-paced. Burn ~3.5us of dummy matmuls on a zeroed
        # tile during the initial DMA wait (results never read).
        warm_sb = per.tile([P, 512], BF16)
        nc.vector.memset(warm_sb, 0.0)

        # ---------------- Phase 1: QT, KT, V projections ----------------
        # The DMA engines round-robin across ALL outstanding descriptors,
        # so every byte issued early delays the critical first pieces, and
        # the Tile scheduler orders instructions by DATA dependencies only
        # (emission order is NOT preserved). Keep only wq+xc0 (the true
        # head dependency, 2MiB) outstanding at the start; every later DMA
        # is gated by a tiny dummy copy INTO its own destination tile
        # (WAW dep) that reads an early xc piece, so it cannot be hoisted.
        # Matmuls run eo-major (4 open psum chains) so each arriving xc
        # piece feeds 4 matmuls immediately during the DMA-paced head.
        with (
            tc.tile_pool(name="wpool", bufs=1) as wpool,
            tc.tile_pool(name="xpool", bufs=2) as xpool,
            tc.tile_pool(name="ps1", bufs=1, space="PSUM") as ps1,
        ):
            wq_sb = wpool.tile([P, EO, HD], BF16)
            wk_sb = wpool.tile([P, EO, HD], BF16)
            wv_sb = wpool.tile([P, EO, HD], BF16)

            PIECES = ((0, 1), (1, 2), (2, 4), (4, 8))
            for lo, hi in PIECES:
                nc.sync.dma_start(
                    out=wq_sb[:, lo:hi, :], in_=wq_d[:, lo * HD : hi * HD]
                )

            xc0 = xpool.tile([P, EO, 512], BF16, tag="xc", name="xc")
            for lo, hi in PIECES:
                nc.gpsimd.dma_start(
                    out=xc0[:, lo:hi, :], in_=xt_d[0, :, lo * 512 : hi * 512]
                )
            # WAW-gated follow-on DMAs (see note above)
            nc.scalar.copy(wk_sb[:, 0, 0:4], xc0[:, 0, 0:4])
            nc.scalar.dma_start(out=wk_sb, in_=wk_d)
            nc.scalar.copy(wv_sb[:, 0, 0:4], xc0[:, 1, 0:4])
            nc.scalar.dma_start(out=wv_sb, in_=wv_d)
            nc.scalar.copy(mask_sb[:, 0, 0:4], xc0[:, 3, 0:4])
            nc.scalar.dma_start(out=mask_sb, in_=mask_d)

            # warmup chain (no data deps -> scheduled before real matmuls)
            ps_w = ps1.tile([P, 512], F32, tag="warm", bufs=1, name="ps_w")
            for i in range(16):
                nc.tensor.matmul(
                    ps_w,
                    lhsT=warm_sb[:, 0:P],
                    rhs=warm_sb,
                    start=(i == 0),
                    stop=(i == 15),
                )

            xc_next = None
            for qc in range(4):  # 512-wide q/k chunk
                xc = xc0 if qc == 0 else xc_next
                if qc < 3:
                    xc_next = xpool.tile([P, EO, 512], BF16, tag="xc", name="xc")
                    nc.scalar.copy(xc_next[:, 0, 0:4], xc[:, 7, 0:4])
                    nc.scalar.dma_start(out=xc_next, in_=xt_d[qc + 1])

                # QT / KT: out[d_tile, q-chunk] = sum_e W[e, d].T @ XT[e, q]
                # dm-major chains: consecutive matmuls share a psum bank
                # (back-to-back matmuls to different banks run ~20% slower).
                # Chunk 0's QT runs split-contraction: all 4 dm chains
                # accumulate eo 0-3 first (those pieces land ~3us before
                # eo 4-7), then finish -- instead of chain dm0 stalling on
                # the last piece while dm1-3 have nothing to do.
                for w_sb, dst, eng in ((wq_sb, qt, "s"), (wk_sb, kt, "v")):
                    if qc == 0 and eng == "s":
                        pss4 = [
                            ps1.tile([P, 512], F32, tag="ps", bufs=5, name=f"p{dm}")
                            for dm in range(DT)
                        ]
                        for dm in range(DT):
                            for eo in range(4):
                                nc.tensor.matmul(
                                    pss4[dm],
                                    lhsT=w_sb[:, eo, dm * P : (dm + 1) * P],
                                    rhs=xc[:, eo, :],
                                    start=(eo == 0),
                                    stop=False,
                                )
                        for dm in range(DT):
                            for eo in range(4, EO):
                                nc.tensor.matmul(
                                    pss4[dm],
                                    lhsT=w_sb[:, eo, dm * P : (dm + 1) * P],
                                    rhs=xc[:, eo, :],
                                    start=False,
                                    stop=(eo == EO - 1),
                                )
                        for dm in range(DT):
                            dslice = dst[:, dm, 0:512]
                            if dm % 2 == 0:
                                nc.scalar.copy(dslice, pss4[dm])
                            else:
                                nc.vector.tensor_copy(dslice, pss4[dm])
                        continue
                    for dm in range(DT):
                        ps = ps1.tile([P, 512], F32, tag="ps", bufs=5)
                        for eo in range(EO):
                            nc.tensor.matmul(
                                ps,
                                lhsT=w_sb[:, eo, dm * P : (dm + 1) * P],
                                rhs=xc[:, eo, :],
                                start=(eo == 0),
                                stop=(eo == EO - 1),
                            )
                        dslice = dst[:, dm, qc * 512 : (qc + 1) * 512]
                        if eng == "s":
                            nc.scalar.copy(dslice, ps)
                        else:
                            nc.vector.tensor_copy(dslice, ps)

                # V: out[k_tile, d] = sum_e XT[e, k].T @ Wv[e, d]
                # psum cols 0:256 -> v[:, kg, 0:256]; 256:512 -> 257:513
                for ki in range(4):
                    kg = qc * 4 + ki
                    ps = ps1.tile([P, 512], F32, tag="ps", bufs=5)
                    for eo in range(EO):
                        nc.tensor.matmul(
                            ps,
                            lhsT=xc[:, eo, ki * P : (ki + 1) * P],
                            rhs=wv_sb[:, eo, :],
                            start=(eo == 0),
                            stop=(eo == EO - 1),
                        )
                    nc.vector.tensor_copy(v[:, kg, 0:256], ps[:, 0:256])
                    nc.scalar.copy(v[:, kg, 257:513], ps[:, 256:512])

        # ---------------- Phase 2: attention ----------------
        with (
            tc.tile_pool(name="apool", bufs=2) as apool,
            tc.tile_pool(name="opool", bufs=2) as opool,
            tc.tile_pool(name="psc", bufs=1, space="PSUM") as psc,
        ):
            for M in range(NSUP):  # q super-tile: q in [512M, 512(M+1))
                at = apool.tile([P, NKT, 512], BF16, tag="at")
                njt = 4 * M + 4  # causal: k-tiles 0 .. 4M+3
                for j in range(njt):
                    r = j - 4 * M
                    # Diagonal-zone tiles: cols < 128r are fully masked.
                    off = P * r if r > 0 else 0
                    ps = pss.tile([P, 512], F32, tag="s")
                    for dt_i in range(DT):
                        nc.tensor.matmul(
                            ps[:, off:512],
                            lhsT=kt[:, dt_i, j * P : (j + 1) * P],
                            rhs=qt[:, dt_i, M * 512 + off : (M + 1) * 512],
                            start=(dt_i == 0),
                            stop=(dt_i == DT - 1),
                        )
                    a_j = at[:, j, off:512]
                    # attnT[k, q] = exp(scoresT / sqrt(hd)); masking after.
                    nc.scalar.activation(a_j, ps[:, off:512], EXP, scale=SCALE)
                    if r >= 0:  # diagonal-zone: zero invalid (q < k) cols
                        nc.vector.tensor_mul(a_j, a_j, mask_sb[:, r, off:512])

                # ctx pass A: cols 0:257 of v_aug (incl. ones column ->
                # denominator in psum col 256). s ascending so s=0 (which
                # doesn't need the last diagonal attn tile) issues first.
                o_sbs = []
                rinvs = []
                for s in range(4):
                    cps = psc.tile([P, 512], F32, tag=f"c{s}", name=f"c{s}")
                    nj = 4 * M + s + 1
                    for j in range(nj):
                        nc.tensor.matmul(
                            cps[:, 0:257],
                            lhsT=at[:, j, s * P : (s + 1) * P],
                            rhs=v[:, j, 0:257],
                            start=(j == 0),
                            stop=(j == nj - 1),
                        )
                    rinv = opool.tile([P, 1], F32, tag=f"r{s}")
                    nc.vector.reciprocal(rinv, cps[:, 256:257])
                    o_sb = opool.tile([P, HD], BF16, tag=f"o{s}")
                    nc.vector.tensor_scalar_mul(
                        o_sb[:, 0:256], cps[:, 0:256], rinv
                    )
                    o_sbs.append(o_sb)
                    rinvs.append(rinv)

                # ctx pass B: cols 257:513 (HD cols 256:512); reuses the
                # same psum banks after pass A's reads complete.
                for s in range(4):
                    cps = psc.tile([P, 512], F32, tag=f"c{s}", name=f"c{s}")
                    nj = 4 * M + s + 1
                    for j in range(nj):
                        nc.tensor.matmul(
                            cps[:, 0:256],
                            lhsT=at[:, j, s * P : (s + 1) * P],
                            rhs=v[:, j, 257:513],
                            start=(j == 0),
                            stop=(j == nj - 1),
                        )
                    nc.vector.tensor_scalar_mul(
                        o_sbs[s][:, 256:512], cps[:, 0:256], rinvs[s]
                    )
                    row0 = M * 512 + s * P
                    nc.sync.dma_start(
                        out=out_d[row0 : row0 + P, :], in_=o_sbs[s]
                    )


def _build_nc():
    nc = bacc.Bacc("TRN2", target_bir_lowering=False, debug=False, num_devices=8)
    # Host-pre-tiled layouts: per-partition lines are contiguous.
    xt_d = nc.dram_tensor("xt", [4, P, EO * 512], BF16, kind="ExternalInput")
    wq_d = nc.dram_tensor("wq", [P, EO * HD], BF16, kind="ExternalInput")
    wk_d = nc.dram_tensor("wk", [P, EO * HD], BF16, kind="ExternalInput")
    wv_d = nc.dram_tensor("wv", [P, EO * HD], BF16, kind="ExternalInput")
    mask_d = nc.dram_tensor("mask", [P, 4, 512], BF16, kind="ExternalInput")
    out_d = nc.dram_tensor("out", [S, HD], BF16, kind="ExternalOutput")
    with tile.TileContext(nc) as tc:
        _body(tc, xt_d.ap(), wq_d.ap(), wk_d.ap(), wv_d.ap(), mask_d.ap(), out_d.ap())
    nc.compile()
    return nc


def _mask_np():
    # mask[p][r, q_local] = 1 iff q_local >= 128*r + k_local(p)
    q = np.arange(512)[None, :]
    k = np.arange(P)[:, None]
    m = np.stack(
        [(q >= (P * r + k)).astype(np.float32) for r in range(4)], axis=0
    )  # [4, 128, 512]
    return np.ascontiguousarray(m.transpose(1, 0, 2))  # [128, 4, 512]


def _in_maps(embedded, Wq, Wk, Wv):
    embedded = np.asarray(embedded, dtype=np.float32)
    mask = _mask_np().astype(ml_dtypes.bfloat16)

    def tile_w(w):  # [E, HD] -> [P, EO*HD] with [p][eo*512+d] layout
        w = np.asarray(w, dtype=np.float32).astype(ml_dtypes.bfloat16)
        return np.ascontiguousarray(
            w.reshape(EO, P, HD).transpose(1, 0, 2).reshape(P, EO * HD)
        )

    def tile_x(x):  # [S, E] -> [4, P, EO*512]: [qc][p][eo*512+col]
        x = np.asarray(x, dtype=np.float32).astype(ml_dtypes.bfloat16)
        return np.ascontiguousarray(
            x.reshape(4, 512, EO, P).transpose(0, 3, 2, 1).reshape(4, P, EO * 512)
        )

    in_maps = []
    for core in range(8):
        b, h = divmod(core, 2)
        in_maps.append(
            {
                "xt": tile_x(embedded[b]),
                "wq": tile_w(Wq[h]),
                "wk": tile_w(Wk[h]),
                "wv": tile_w(Wv[h]),
                "mask": mask,
            }
        )
    return in_maps


def _gather(results):
    out = np.empty((B, S, H * HD), np.float32)
    for core in range(8):
        b, h = divmod(core, 2)
        out[b, :, h * HD : (h + 1) * HD] = np.asarray(
            results[core]["out"]
        ).astype(np.float32)
    return out


def _get_nc():
    global _NC
    if _NC is None:
        _NC = _build_nc()
    return _NC


def kernel(embedded, Wq, Wk, Wv):
    res = bass_utils.run_bass_kernel_spmd(
        _get_nc(), _in_maps(embedded, Wq, Wk, Wv), core_ids=list(range(8))
    )
    return _gather(res.results)


def kernel_traced(embedded, Wq, Wk, Wv):
    """Like kernel() but with NTFF tracing; returns (out, BassKernelResults)."""
    res = bass_utils.run_bass_kernel_spmd(
        _get_nc(), _in_maps(embedded, Wq, Wk, Wv), core_ids=list(range(8)), trace=True
    )
    return _gather(res.results), res


# revision 14
# speedup vs baseline: 1.1844x; 1.0160x over previous
"""Trainium2 Bass kernel for 2-head causal self-attention.

Problem: embedded [B=4, S=2048, E=1024], Wq/Wk/Wv [H=2, E, HD=512].
out[b, s, h*HD:(h+1)*HD] = softmax(causal(Q K^T / sqrt(HD))) @ V for head h.

Sharding: 8 (b, h) pairs -> 8 cores, one pair each (perfect SPMD balance).

Per-core dataflow (all matmul operands bf16, PSUM f32; 1 cyc/row):
  - Host pre-tiles X^T and W into SBUF-layout DRAM buffers so every DMA
    descriptor has 1-8 KB contiguous per-partition lines (the baseline's
    1KB-line packets capped aggregate DMA at ~72 GB/s and starved the
    pipeline head).
  - Phase 1: QT[d,q], KT[d,q] (W stationary, X^T moving) and V[k,d]
    (X^T stationary, W moving), q/k pipelined in 512-wide chunks.
    V is stored augmented with a ones column at index 256:
    v_aug[k, 0:256]=V[:,0:256], v_aug[k,256]=1, v_aug[k,257:513]=V[:,256:512].
  - Phase 2: scores computed TRANSPOSED: scoresT[k, q-chunk] =
    (KT tile).T @ QT; exp on scalar engine, 0/1 diagonal-block mask on
    vector engine. ctx[q_sub, :] = sum_j attnT[:, q_sub].T @ v_aug[:, j]
    in TWO chains per q-subtile: cols 0:257 (incl. ones column -> the
    softmax denominator lands in psum col 256 as a per-partition scalar)
    and cols 257:513. This removes the baseline's 40 ones-matmul row-sum
    instructions (~17.9k PE cycles) and its 16 rotate matmuls entirely.
  - Strictly-above-diagonal blocks skipped; diagonal-zone score matmuls
    start at column off=128*r (bf16 has no fp32r >=256-width constraint).
  - Output written bf16 (host upcasts); halves output DMA.
"""

import ml_dtypes
import numpy as np

import concourse.bass as bass
import concourse.mybir as mybir
from concourse import bacc
import concourse.tile as tile
from concourse import bass_utils

B, S, E, H, HD = 4, 2048, 1024, 2, 512
P = 128
EO = E // P          # 8 e-tiles (contraction for QKV)
DT = HD // P         # 4 d-tiles (contraction for scores)
NKT = S // P         # 16 k-tiles
NSUP = S // 512      # 4 q super-tiles (512 wide)
HDA = HD + 1         # 513: V augmented with ones column at 256
SCALE = float(HD) ** -0.5
F32 = mybir.dt.float32
EXP = mybir.ActivationFunctionType.Exp
BF16 = mybir.dt.bfloat16

_NC = None


def _body(tc, xt_d, wq_d, wk_d, wv_d, mask_d, out_d):
    nc = tc.nc

    import contextlib

    with contextlib.ExitStack() as ctx:
        per = ctx.enter_context(tc.tile_pool(name="per", bufs=1))
        # Persistent SBUF: QT/KT as [d_inner=128, d_tile, q], V as
        # [k_inner, k_tile, 513], mask diagonal patterns.
        qt = per.tile([P, DT, S], BF16)
        kt = per.tile([P, DT, S], BF16)
        v = per.tile([P, NKT, HDA], BF16)
        mask_sb = per.tile([P, 4, 512], BF16)

        # ones column of v_aug (written once, before any V psum copy).
        # On vector, NOT gpsimd: gpsimd must issue the first xc DMA
        # descriptors as early as possible.
        nc.vector.memset(v[:, :, 256:257], 1.0)

        # Scores psum pool opened FIRST so it gets PSUM banks phase 1
        # never touches: phase 2's first scores chain then has no
        # write-after-read wait on a phase-1 bank.
        pss = ctx.enter_context(tc.tile_pool(name="pss", bufs=2, space="PSUM"))

        # PE p-state warmup: the tensor engine clock ramps with sustained
        # use, and the first real matmuls otherwise run ~60% slower while
        # also being DMA-paced. Burn ~3.5us of dummy matmuls on a zeroed
        # tile during the initial DMA wait (results never read).
        warm_sb = per.tile([P, 512], BF16)
        nc.vector.memset(warm_sb, 0.0)

        # ---------------- Phase 1: QT, KT, V projections ----------------
        # The DMA engines round-robin across ALL outstanding descriptors,
        # so every byte issued early delays the critical first pieces, and
        # the Tile scheduler orders instructions by DATA dependencies only
        # (emission order is NOT preserved). Keep only wq+xc0 (the true
        # head dependency, 2MiB) outstanding at the start; every later DMA
        # is gated by a tiny dummy copy INTO its own destination tile
        # (WAW dep) that reads an early xc piece, so it cannot be hoisted.
        # Matmuls run eo-major (4 open psum chains) so each arriving xc
        # piece feeds 4 matmuls immediately during the DMA-paced head.
        with (
            tc.tile_pool(name="wpool", bufs=1) as wpool,
            tc.tile_pool(name="xpool", bufs=2) as xpool,
            tc.tile_pool(name="ps1", bufs=1, space="PSUM") as ps1,
        ):
            wq_sb = wpool.tile([P, EO, HD], BF16)
            wk_sb = wpool.tile([P, EO, HD], BF16)
            wv_sb = wpool.tile([P, EO, HD], BF16)

            PIECES = ((0, 1), (1, 2), (2, 3), (3, 4), (4, 8))
            for lo, hi in PIECES:
                nc.sync.dma_start(
                    out=wq_sb[:, lo:hi, :], in_=wq_d[:, lo * HD : hi * HD]
                )

            xc0 = xpool.tile([P, EO, 512], BF16, tag="xc", name="xc")
            for lo, hi in PIECES:
                nc.gpsimd.dma_start(
                    out=xc0[:, lo:hi, :], in_=xt_d[0, :, lo * 512 : hi * 512]
                )
            # WAW-gated follow-on DMAs (see note above)
            nc.scalar.copy(wk_sb[:, 0, 0:4], xc0[:, 1, 0:4])
            nc.scalar.dma_start(out=wk_sb, in_=wk_d)
            nc.scalar.copy(wv_sb[:, 0, 0:4], xc0[:, 3, 0:4])
            nc.scalar.dma_start(out=wv_sb, in_=wv_d)
            nc.scalar.copy(mask_sb[:, 0, 0:4], xc0[:, 7, 0:4])
            nc.scalar.dma_start(out=mask_sb, in_=mask_d)

            # warmup chain (no data deps -> scheduled before real matmuls)
            ps_w = ps1.tile([P, 512], F32, tag="warm", bufs=1, name="ps_w")
            for i in range(7):
                nc.tensor.matmul(
                    ps_w,
                    lhsT=warm_sb[:, 0:P],
                    rhs=warm_sb,
                    start=(i == 0),
                    stop=(i == 6),
                )

            xc_next = None
            for qc in range(4):  # 512-wide q/k chunk
                xc = xc0 if qc == 0 else xc_next
                if qc < 3:
                    xc_next = xpool.tile([P, EO, 512], BF16, tag="xc", name="xc")
                    nc.scalar.copy(xc_next[:, 0, 0:4], xc[:, 7, 0:4])
                    nc.scalar.dma_start(out=xc_next, in_=xt_d[qc + 1])

                # QT / KT: out[d_tile, q-chunk] = sum_e W[e, d].T @ XT[e, q]
                # dm-major chains: consecutive matmuls share a psum bank
                # (back-to-back matmuls to different banks run ~20% slower).
                # Chunk 0's QT runs split-contraction: all 4 dm chains
                # accumulate eo 0-3 first (those pieces land ~3us before
                # eo 4-7), then finish -- instead of chain dm0 stalling on
                # the last piece while dm1-3 have nothing to do.
                for w_sb, dst, eng in ((wq_sb, qt, "s"), (wk_sb, kt, "v")):
                    if qc == 0 and eng == "s":
                        pss4 = [
                            ps1.tile([P, 512], F32, tag="ps", bufs=5, name=f"p{dm}")
                            for dm in range(DT)
                        ]
                        for eo in range(4):
                            for dm in range(DT):
                                nc.tensor.matmul(
                                    pss4[dm],
                                    lhsT=w_sb[:, eo, dm * P : (dm + 1) * P],
                                    rhs=xc[:, eo, :],
                                    start=(eo == 0),
                                    stop=False,
                                )
                        for dm in range(DT):
                            for eo in range(4, EO):
                                nc.tensor.matmul(
                                    pss4[dm],
                                    lhsT=w_sb[:, eo, dm * P : (dm + 1) * P],
                                    rhs=xc[:, eo, :],
                                    start=False,
                                    stop=(eo == EO - 1),
                                )
                        for dm in range(DT):
                            dslice = dst[:, dm, 0:512]
                            if dm % 2 == 0:
                                nc.scalar.copy(dslice, pss4[dm])
                            else:
                                nc.vector.tensor_copy(dslice, pss4[dm])
                        continue
                    for dm in range(DT):
                        ps = ps1.tile([P, 512], F32, tag="ps", bufs=5)
                        for eo in range(EO):
                            nc.tensor.matmul(
                                ps,
                                lhsT=w_sb[:, eo, dm * P : (dm + 1) * P],
                                rhs=xc[:, eo, :],
                                start=(eo == 0),
                                stop=(eo == EO - 1),
                            )
                        dslice = dst[:, dm, qc * 512 : (qc + 1) * 512]
                        if eng == "s":
                            nc.scalar.copy(dslice, ps)
                        else:
                            nc.vector.tensor_copy(dslice, ps)

                # V: out[k_tile, d] = sum_e XT[e, k].T @ Wv[e, d]
                # psum cols 0:256 -> v[:, kg, 0:256]; 256:512 -> 257:513
                for ki in range(4):
                    kg = qc * 4 + ki
                    ps = ps1.tile([P, 512], F32, tag="ps", bufs=5)
                    for eo in range(EO):
                        nc.tensor.matmul(
                            ps,
                            lhsT=xc[:, eo, ki * P : (ki + 1) * P],
                            rhs=wv_sb[:, eo, :],
                            start=(eo == 0),
                            stop=(eo == EO - 1),
                        )
                    nc.vector.tensor_copy(v[:, kg, 0:256], ps[:, 0:256])
                    nc.scalar.copy(v[:, kg, 257:513], ps[:, 256:512])

        # ---------------- Phase 2: attention ----------------
        with (
            tc.tile_pool(name="apool", bufs=2) as apool,
            tc.tile_pool(name="opool", bufs=2) as opool,
            tc.tile_pool(name="psc", bufs=1, space="PSUM") as psc,
        ):
            for M in range(NSUP):  # q super-tile: q in [512M, 512(M+1))
                at = apool.tile([P, NKT, 512], BF16, tag="at")
                njt = 4 * M + 4  # causal: k-tiles 0 .. 4M+3
                for j in range(njt):
                    r = j - 4 * M
                    # Diagonal-zone tiles: cols < 128r are fully masked.
                    off = P * r if r > 0 else 0
                    ps = pss.tile([P, 512], F32, tag="s")
                    for dt_i in range(DT):
                        nc.tensor.matmul(
                            ps[:, off:512],
                            lhsT=kt[:, dt_i, j * P : (j + 1) * P],
                            rhs=qt[:, dt_i, M * 512 + off : (M + 1) * 512],
                            start=(dt_i == 0),
                            stop=(dt_i == DT - 1),
                        )
                    a_j = at[:, j, off:512]
                    # attnT[k, q] = exp(scoresT / sqrt(hd)); masking after.
                    nc.scalar.activation(a_j, ps[:, off:512], EXP, scale=SCALE)
                    if r >= 0:  # diagonal-zone: zero invalid (q < k) cols
                        nc.vector.tensor_mul(a_j, a_j, mask_sb[:, r, off:512])

                # ctx pass A: cols 0:257 of v_aug (incl. ones column ->
                # denominator in psum col 256). s ascending so s=0 (which
                # doesn't need the last diagonal attn tile) issues first.
                o_sbs = []
                rinvs = []
                for s in range(4):
                    cps = psc.tile([P, 512], F32, tag=f"c{s}", name=f"c{s}")
                    nj = 4 * M + s + 1
                    for j in range(nj):
                        nc.tensor.matmul(
                            cps[:, 0:257],
                            lhsT=at[:, j, s * P : (s + 1) * P],
                            rhs=v[:, j, 0:257],
                            start=(j == 0),
                            stop=(j == nj - 1),
                        )
                    rinv = opool.tile([P, 1], F32, tag=f"r{s}")
                    nc.vector.reciprocal(rinv, cps[:, 256:257])
                    o_sb = opool.tile([P, HD], BF16, tag=f"o{s}")
                    nc.vector.tensor_scalar_mul(
                        o_sb[:, 0:256], cps[:, 0:256], rinv
                    )
                    o_sbs.append(o_sb)
                    rinvs.append(rinv)

                # ctx pass B: cols 257:513 (HD cols 256:512); reuses the
                # same psum banks after pass A's reads complete.
                for s in range(4):
                    cps = psc.tile([P, 512], F32, tag=f"c{s}", name=f"c{s}")
                    nj = 4 * M + s + 1
                    for j in range(nj):
                        nc.tensor.matmul(
                            cps[:, 0:256],
                            lhsT=at[:, j, s * P : (s + 1) * P],
                            rhs=v[:, j, 257:513],
                            start=(j == 0),
                            stop=(j == nj - 1),
                        )
                    nc.vector.tensor_scalar_mul(
                        o_sbs[s][:, 256:512], cps[:, 0:256], rinvs[s]
                    )
                    row0 = M * 512 + s * P
                    nc.sync.dma_start(
                        out=out_d[row0 : row0 + P, :], in_=o_sbs[s]
                    )


def _build_nc():
    nc = bacc.Bacc("TRN2", target_bir_lowering=False, debug=False, num_devices=8)
    # Host-pre-tiled layouts: per-partition lines are contiguous.
    xt_d = nc.dram_tensor("xt", [4, P, EO * 512], BF16, kind="ExternalInput")
    wq_d = nc.dram_tensor("wq", [P, EO * HD], BF16, kind="ExternalInput")
    wk_d = nc.dram_tensor("wk", [P, EO * HD], BF16, kind="ExternalInput")
    wv_d = nc.dram_tensor("wv", [P, EO * HD], BF16, kind="ExternalInput")
    mask_d = nc.dram_tensor("mask", [P, 4, 512], BF16, kind="ExternalInput")
    out_d = nc.dram_tensor("out", [S, HD], BF16, kind="ExternalOutput")
    with tile.TileContext(nc) as tc:
        _body(tc, xt_d.ap(), wq_d.ap(), wk_d.ap(), wv_d.ap(), mask_d.ap(), out_d.ap())
    nc.compile()
    return nc


def _mask_np():
    # mask[p][r, q_local] = 1 iff q_local >= 128*r + k_local(p)
    q = np.arange(512)[None, :]
    k = np.arange(P)[:, None]
    m = np.stack(
        [(q >= (P * r + k)).astype(np.float32) for r in range(4)], axis=0
    )  # [4, 128, 512]
    return np.ascontiguousarray(m.transpose(1, 0, 2))  # [128, 4, 512]


def _in_maps(embedded, Wq, Wk, Wv):
    embedded = np.asarray(embedded, dtype=np.float32)
    mask = _mask_np().astype(ml_dtypes.bfloat16)

    def tile_w(w):  # [E, HD] -> [P, EO*HD] with [p][eo*512+d] layout
        w = np.asarray(w, dtype=np.float32).astype(ml_dtypes.bfloat16)
        return np.ascontiguousarray(
            w.reshape(EO, P, HD).transpose(1, 0, 2).reshape(P, EO * HD)
        )

    def tile_x(x):  # [S, E] -> [4, P, EO*512]: [qc][p][eo*512+col]
        x = np.asarray(x, dtype=np.float32).astype(ml_dtypes.bfloat16)
        return np.ascontiguousarray(
            x.reshape(4, 512, EO, P).transpose(0, 3, 2, 1).reshape(4, P, EO * 512)
        )

    in_maps = []
    for core in range(8):
        b, h = divmod(core, 2)
        in_maps.append(
            {
                "xt": tile_x(embedded[b]),
                "wq": tile_w(Wq[h]),
                "wk": tile_w(Wk[h]),
                "wv": tile_w(Wv[h]),
                "mask": mask,
            }
        )
    return in_maps


def _gather(results):
    out = np.empty((B, S, H * HD), np.float32)
    for core in range(8):
        b, h = divmod(core, 2)
        out[b, :, h * HD : (h + 1) * HD] = np.asarray(
            results[core]["out"]
        ).astype(np.float32)
    return out


def _get_nc():
    global _NC
    if _NC is None:
        _NC = _build_nc()
    return _NC


def kernel(embedded, Wq, Wk, Wv):
    res = bass_utils.run_bass_kernel_spmd(
        _get_nc(), _in_maps(embedded, Wq, Wk, Wv), core_ids=list(range(8))
    )
    return _gather(res.results)


def kernel_traced(embedded, Wq, Wk, Wv):
    """Like kernel() but with NTFF tracing; returns (out, BassKernelResults)."""
    res = bass_utils.run_bass_kernel_spmd(
        _get_nc(), _in_maps(embedded, Wq, Wk, Wv), core_ids=list(range(8)), trace=True
    )
    return _gather(res.results), res
